# revision 30
# baseline (speedup 1.0000x reference)
"""Trainium2 Bass kernel for nn_CropConvLSTM.

Model: ConvLSTM (Conv1d(1+H -> 4H, k=3, pad=1), S=12 steps) over x (B=256,
S=12, L=128), then head Linear(98304->768)+BN+ReLU, Linear(768->12)+BN+ReLU,
Linear(12->10).

Distribution over 8 NeuronCores, three launches:
  Stage 1: ConvLSTM, data-parallel over batch (32 samples/core). Conv done as
    3 shifted fp32r matmuls (K=66: 64 h-channels + x-row + ones-row for the
    conv bias) accumulating in PSUM; gates on ACT/DVE/GPSIMD with all tensors
    at legal partition bases (tanh computed as 2*sigmoid(2x)-1 so a single
    per-partition-scaled sigmoid covers the [o;g] psum tile).
  Stage 2: y1 = flat @ (w1*bn1_scale).T, sharded over the 98304 contract dim
    (12288 features/core); each core emits a partial (768, 256), host reduces.
  Stage 3: bias+ReLU, Linear2+BN+ReLU, Linear3 (+b3 via ones-row trick),
    data-parallel over batch again.

BN (eval mode) is folded into the weights/biases on the host.
"""
import os
import sys

sys.path.insert(0, "/opt/trn_rl_repo")

from functools import lru_cache

import numpy as np

import concourse.bass as bass
import concourse.tile as tile
from concourse import bacc, mybir
from concourse.bass_utils import run_bass_kernel_spmd

F32 = mybir.dt.float32
F32R = mybir.dt.float32r
BF16 = mybir.dt.bfloat16
AF = mybir.ActivationFunctionType

B, S, L, H, C = 256, 12, 128, 64, 10
NC = 8
BLOC = B // NC            # 32 samples per core in stages 1/3
KTOT = S * H * L          # 98304
KSH = KTOT // NC          # 12288 contract features per core in stage 2
KCH = KSH // 128          # 96 k-chunks per core
EPS = 1e-5
CORE_IDS = list(range(NC))


# ---------------------------------------------------------------- stage 1
#
# Layout (per core, 32 samples):
#   T1 [128, 32, L+2] bf16: rows 0:64  = h stored at col j+1 (tap0 window)
#                           rows 64:128= h stored at col j   (tap1 window)
#   T2 [68, 32, L+2]  bf16: rows 0:64  = h stored at col j-1 (tap2 window)
#                           rows 64:67 = x at shifts +1/0/-1, row 67 = ones
#   Conv per step = TWO matmul passes per (quarter, gate-pair):
#     pass A: K=128, rhs=T1[:, q, 0:L]   (h taps 0+1)
#     pass B: K=68,  rhs=T2[:, q, 0:L]   (h tap2 + x*3 + bias)
#   Shifted h copies are SBUF->SBUF DMAs off the critical engines.
#   Gates: sigmoid drains on ACT ([f;i] plain, [o;g] with per-partition
#   scale [1;2]), tanh(g) unfold 2s-1 on Pool, state math on DVE in bf16
#   (2x mode), tanh(c) native on ACT over both halves packed 128-part.
QS = 8                    # samples per quarter
NQ = BLOC // QS           # 4 quarters
HB = BLOC // 2            # 16 samples per half (c packing)


@lru_cache(maxsize=1)
def _build_stage1():
    nc = bacc.Bacc("TRN2", target_bir_lowering=False, debug=False, num_devices=NC)
    xr = nc.dram_tensor("xr", [S, 3, BLOC, L + 2], BF16, kind="ExternalInput").ap()
    ones = nc.dram_tensor("ones", [1, BLOC, L + 2], BF16, kind="ExternalInput").ap()
    # gate column order per half: f/o at the half's c-partition base, i/g at
    # the other (HW requires equal base partitions for SBUF x SBUF tensor ops)
    wabs = {
        nm: nc.dram_tensor(nm, [dim, 128], BF16, kind="ExternalInput").ap()
        for hf in range(2)
        for nm, dim in ((f"wa_fi{hf}", 128), (f"wb_fi{hf}", 68),
                        (f"wa_og{hf}", 128), (f"wb_og{hf}", 68))
    }
    hs = nc.dram_tensor("hs", [S, H, BLOC, L], BF16, kind="ExternalOutput").ap()

    with tile.TileContext(nc) as tc:
        with (
            tc.tile_pool(name="persist", bufs=1) as pp,
            tc.tile_pool(name="sig", bufs=6) as sp,
            tc.tile_pool(name="tmp", bufs=4) as tp,
            tc.tile_pool(name="psfi", bufs=2, space="PSUM") as ps_fi,
            tc.tile_pool(name="psog", bufs=2, space="PSUM") as ps_og,
        ):
            t1h = pp.tile([128, BLOC, L + 2], BF16)
            # T2 is double-buffered by step parity: x rows for step s+1
            # prefetch into the other buffer, and the h tap2 copy of step s
            # goes into the buffer used by step s+1. This keeps the per-step
            # x DMA off the inter-step critical path.
            t2hs = [pp.tile([68, BLOC, L + 2], BF16, name=f"t2h{i}")
                    for i in range(2)]
            cpk = pp.tile([128, HB, L], BF16)           # c: [half0 | half1]
            wts = {}
            for hf in range(2):
                for nm, dim in ((f"wa_fi{hf}", 128), (f"wb_fi{hf}", 68),
                                (f"wa_og{hf}", 128), (f"wb_og{hf}", 68)):
                    wts[nm] = pp.tile([dim, 128], BF16, name=f"t_{nm}")
            scv0 = pp.tile([128, 1], F32)               # og act scale [1;2]
            scv1 = pp.tile([128, 1], F32)               # og act scale [2;1]

            nc.vector.memset(t1h, 0.0)
            nc.vector.memset(t2hs[0], 0.0)
            nc.vector.memset(t2hs[1], 0.0)
            nc.vector.memset(cpk, 0.0)
            nc.vector.memset(scv0[0:64], 1.0)
            nc.vector.memset(scv0[64:128], 2.0)
            nc.vector.memset(scv1[0:64], 2.0)
            nc.vector.memset(scv1[64:128], 1.0)
            nc.sync.dma_start(out=t2hs[0][67:68], in_=ones)
            nc.sync.dma_start(out=t2hs[1][67:68], in_=ones)
            for nm, tl in wts.items():
                nc.sync.dma_start(out=tl, in_=wabs[nm])
            nc.sync.dma_start(out=t2hs[0][64:67], in_=xr[0])

            # --- software pipeline helpers -------------------------------
            def front(s, q, so_tiles):
                """matmuls + gate sigmoids + c update for one quarter.

                For half hf, f and o land at partitions [64hf:64hf+64] (the
                half's c base: "pri"), i and g at the other 64 ("alt") --
                SBUF x SBUF tensor ops need equal input base partitions.
                """
                t2h = t2hs[s % 2]
                hf, qs = q // 2, (q % 2) * QS
                pri = slice(64 * hf, 64 * hf + 64)
                alt = slice(64 - 64 * hf, 128 - 64 * hf)
                smp = slice(q * QS, q * QS + QS)
                pfi = ps_fi.tile([128, QS, L], F32, name="pfi")
                pog = ps_og.tile([128, QS, L], F32, name="pog")
                # og first everywhere: it feeds the long chain
                # og -> tanh(g) -> t2 -> c -> tanh(c) -> h
                # matmul outputs must fit one PSUM bank (512 f32), so each
                # pass is issued per 4-sample half of the 8-sample tile
                for G, ps in (("og", pog), ("fi", pfi)):
                    wa, wb = wts[f"wa_{G}{hf}"], wts[f"wb_{G}{hf}"]
                    for hb in range(2):
                        sm4 = slice(q * QS + 4 * hb, q * QS + 4 * hb + 4)
                        ob = ps[:, 4 * hb : 4 * hb + 4, :]
                        nc.tensor.matmul(ob, lhsT=wa,
                                         rhs=t1h[:, sm4, 0:L],
                                         start=True, stop=False)
                        nc.tensor.matmul(ob, lhsT=wb,
                                         rhs=t2h[:, sm4, 0:L],
                                         start=False, stop=True)

                sg = sp.tile([128, QS, L], BF16, name="sg")
                # so stays f32: the 2s-1 unfold would amplify bf16 rounding
                # of sig(2g) into absolute tanh(g) error
                so = sp.tile([128, QS, L], F32, name="so")
                so_tiles[q] = so
                tg = tp.tile([128, QS, L], BF16, name="tg")
                # so = [sig(o)@pri; sig(2g)@alt], sg = [sig(f)@pri; sig(i)@alt]
                nc.scalar.activation(so, pog, AF.Sigmoid,
                                     scale=scv0 if hf == 0 else scv1)
                nc.scalar.activation(sg, pfi, AF.Sigmoid)
                # tanh(g) = 2*sig(2g) - 1  (Pool engine)
                nc.gpsimd.tensor_scalar(
                    out=tg[alt], in0=so[alt], scalar1=2.0, scalar2=-1.0,
                    op0=mybir.AluOpType.mult, op1=mybir.AluOpType.add,
                )
                # c = sig(f)*c + sig(i)*tanh(g)   (DVE, bf16 2x)
                cq = cpk[pri, qs : qs + QS, :]
                t1t = tp.tile([128, QS, L], BF16, name="t1t")
                t2t = tp.tile([128, QS, L], BF16, name="t2t")
                nc.vector.tensor_mul(t1t[pri], sg[pri], cq)
                nc.vector.tensor_mul(t2t[pri], sg[alt], tg[alt])
                nc.vector.tensor_add(cq, t1t[pri], t2t[pri])

            def tail(s, pair, so_tiles):
                """tanh(c), h writes, shifted copies and hs store."""
                t2n = t2hs[(s + 1) % 2]      # tap2 copy feeds step s+1
                qs = pair * QS
                tcs = tp.tile([128, QS, L], BF16, name="tcs")
                nc.scalar.activation(tcs, cpk[:, qs : qs + QS, :], AF.Tanh)
                for q in (pair, pair + 2):
                    hf = q // 2
                    pri = slice(64 * hf, 64 * hf + 64)
                    smp = slice(q * QS, q * QS + QS)
                    soq = so_tiles[q]
                    # h = sig(o) * tanh(c) -> T1 rows 0:64 at col shift +1
                    nc.vector.tensor_mul(
                        t1h[0:64, smp, 1 : L + 1], soq[pri], tcs[pri],
                    )
                    # shifted copies for tap1/tap2 windows
                    nc.sync.dma_start(
                        out=t1h[64:128, smp, 0:L],
                        in_=t1h[0:64, smp, 1 : L + 1],
                    )
                    nc.sync.dma_start(
                        out=t2n[0:64, smp, 0:L],
                        in_=t1h[0:64, smp, 2 : L + 2],
                    )
                    nc.sync.dma_start(
                        out=hs[s, :, smp, :],
                        in_=t1h[0:64, smp, 1 : L + 1],
                    )

            # pair-slot pipeline: tail of the previous pair issues BETWEEN
            # the two quarter-fronts of the current pair, so every engine
            # stream stays dependency-ordered without head blocking
            pending = None
            for s in range(S):
                # prefetch next step's x rows into the other T2 buffer
                if s + 1 < S:
                    nc.sync.dma_start(out=t2hs[(s + 1) % 2][64:67],
                                      in_=xr[s + 1])
                for pair in range(2):
                    so_tiles = {}
                    front(s, pair, so_tiles)
                    if pending is not None:
                        tail(*pending)
                    front(s, pair + 2, so_tiles)
                    pending = (s, pair, so_tiles)
            tail(*pending)
    nc.compile()
    return nc


# ---------------------------------------------------------------- stage 2
@lru_cache(maxsize=1)
def _build_stage2():
    nc = bacc.Bacc("TRN2", target_bir_lowering=False, debug=False, num_devices=NC)
    w1p = nc.dram_tensor("w1p", [KCH, 128, 768], BF16, kind="ExternalInput").ap()
    ft = nc.dram_tensor("ft", [KCH, 128, B], BF16, kind="ExternalInput").ap()
    y1p = nc.dram_tensor("y1p", [768, B], F32, kind="ExternalOutput").ap()

    KB = 2                       # k-chunks per DMA batch
    NB = KCH // KB               # 48 batches

    with tile.TileContext(nc) as tc:
        with (
            tc.tile_pool(name="wp", bufs=4) as wp,
            tc.tile_pool(name="rp", bufs=4) as rp,
            tc.tile_pool(name="op", bufs=2) as op,
            tc.tile_pool(name="ps", bufs=1, space="PSUM") as ps,
        ):
            acc = [ps.tile([128, B], F32, name=f"acc{m}") for m in range(6)]
            for kb in range(NB):
                wt = wp.tile([128, KB, 768], BF16, name="wt")
                rt = rp.tile([128, KB, B], BF16, name="rt")
                nc.sync.dma_start(
                    out=wt,
                    in_=w1p[kb * KB : (kb + 1) * KB].rearrange("k p m -> p k m"),
                )
                nc.gpsimd.dma_start(
                    out=rt,
                    in_=ft[kb * KB : (kb + 1) * KB].rearrange("k p b -> p k b"),
                )
                last_kb = kb == NB - 1
                for kc in range(KB):
                    last = last_kb and kc == KB - 1
                    for m in range(6):
                        nc.tensor.matmul(
                            acc[m], lhsT=wt[:, kc, m * 128 : (m + 1) * 128],
                            rhs=rt[:, kc, :],
                            start=(kb == 0 and kc == 0),
                            stop=last,
                        )
                        if last:
                            # drain this M-tile immediately after its final
                            # accumulation to overlap the output tail
                            ot = op.tile([128, B], F32, name="ot")
                            nc.vector.tensor_copy(ot, acc[m])
                            nc.sync.dma_start(
                                out=y1p[m * 128 : (m + 1) * 128], in_=ot,
                            )
    nc.compile()
    return nc


# ---------------------------------------------------------------- stage 3
# One packed const tensor, ReLUs on DVE (no ACT table load), 2 input DMAs.
# cst cols: 0:6 = c1 (bias1, transposed 6x128), 6:78 = w2 (6 k-chunks of
# (128,12)), 78:79 = c2 (partitions 0:12), 79:89 = [w3.T; b3] (parts 0:13)
@lru_cache(maxsize=1)
def _build_stage3():
    nc = bacc.Bacc("TRN2", target_bir_lowering=False, debug=False, num_devices=NC)
    y1s = nc.dram_tensor("y1s", [128, 6, BLOC], F32R, kind="ExternalInput").ap()
    cstv = nc.dram_tensor("cstv", [128, 7], F32, kind="ExternalInput").ap()
    cstw = nc.dram_tensor("cstw", [128, 82], F32R, kind="ExternalInput").ap()
    y3p = nc.dram_tensor("y3p", [BLOC, C], F32, kind="ExternalOutput").ap()
    MX = mybir.AluOpType.max
    AD = mybir.AluOpType.add

    with tile.TileContext(nc) as tc:
        with (
            tc.tile_pool(name="sb", bufs=1) as sb,
            tc.tile_pool(name="ps", bufs=1, space="PSUM") as ps,
        ):
            yt = sb.tile([128, 6, BLOC], F32R)
            cv = sb.tile([128, 7], F32)
            cw = sb.tile([128, 82], F32R)
            nc.sync.dma_start(out=cv, in_=cstv)
            nc.sync.dma_start(out=cw, in_=cstw)
            nc.sync.dma_start(out=yt, in_=y1s)

            r1 = sb.tile([128, 6, BLOC], F32R)
            p2 = ps.tile([12, BLOC], F32)
            for kc in range(6):
                # r1 = relu(y1 + c1)  (DVE, per-partition bias)
                nc.vector.tensor_scalar(
                    out=r1[:, kc, :], in0=yt[:, kc, :],
                    scalar1=cv[:, kc : kc + 1], scalar2=0.0, op0=AD, op1=MX,
                )
                nc.tensor.matmul(
                    p2, lhsT=cw[:, 12 * kc : 12 * kc + 12],
                    rhs=r1[:, kc, :], start=(kc == 0), stop=(kc == 5),
                )
            r2 = sb.tile([13, BLOC], F32R)
            # ones row lives at partition 12 (not 32-aligned): fill the whole
            # tile with 1.0 first, then overwrite rows 0..11
            nc.vector.memset(r2.bitcast(F32), 1.0)
            nc.vector.tensor_scalar(
                out=r2[0:12], in0=p2, scalar1=cv[0:12, 6:7], scalar2=0.0,
                op0=AD, op1=MX,
            )
            p3 = ps.tile([BLOC, C], F32)
            nc.tensor.matmul(p3, lhsT=r2, rhs=cw[0:13, 72:82],
                             start=True, stop=True)
            ot = sb.tile([BLOC, C], F32)
            nc.vector.tensor_copy(ot, p3)
            nc.sync.dma_start(out=y3p, in_=ot)
    nc.compile()
    return nc


# ---------------------------------------------------------------- host glue
def _prep_stage1_inputs(x, conv_w, conv_b):
    """Per-core stage-1 in_maps. conv_w: (4H, 1+H, 3), in-ch 0 = x, 1.. = h.
    Gate order in conv_w rows: i, f, o, g (64 each). Our gate tiles are
    [f; i] and [o; g]. lhsT layouts:
      wa_G [128, 128]: K rows 0:64 = h tap0, 64:128 = h tap1
      wb_G [68, 128]:  K rows 0:64 = h tap2, 64:67 = x taps 0/1/2, 67 = bias
    """
    import ml_dtypes
    bf = ml_dtypes.bfloat16
    f32 = np.float32
    w = np.asarray(conv_w, f32)
    b = np.asarray(conv_b, f32)
    gi, gf, go, gg = w[0:64], w[64:128], w[128:192], w[192:256]
    bi, bf_, bo, bg = b[0:64], b[64:128], b[128:192], b[192:256]

    def lhst(wA, wB, bA, bB):
        wa = np.zeros((128, 128), f32)
        wb = np.zeros((68, 128), f32)
        for col0, wg_, bg_ in ((0, wA, bA), (64, wB, bB)):
            # wg_: (64 out-ch, 65 in-ch, 3 taps); h = in-ch 1.., x = in-ch 0
            wa[0:64, col0 : col0 + 64] = wg_[:, 1:65, 0].T
            wa[64:128, col0 : col0 + 64] = wg_[:, 1:65, 1].T
            wb[0:64, col0 : col0 + 64] = wg_[:, 1:65, 2].T
            wb[64:67, col0 : col0 + 64] = wg_[:, 0, :].T
            wb[67, col0 : col0 + 64] = bg_
        return wa.astype(bf), wb.astype(bf)

    wd = {}
    # half 0: [f|i], [o|g]; half 1: [i|f], [g|o] (f/o at the half's c base)
    wd["wa_fi0"], wd["wb_fi0"] = lhst(gf, gi, bf_, bi)
    wd["wa_og0"], wd["wb_og0"] = lhst(go, gg, bo, bg)
    wd["wa_fi1"], wd["wb_fi1"] = lhst(gi, gf, bi, bf_)
    wd["wa_og1"], wd["wb_og1"] = lhst(gg, go, bg, bo)

    ones = np.ones((1, BLOC, L + 2), f32).astype(bf)
    maps = []
    for c in range(NC):
        xc = np.asarray(x[c * BLOC : (c + 1) * BLOC].transpose(1, 0, 2), f32)
        # xr[s]: rows at shifts +1 / 0 / -1 over L+2 cols (window = cols 0:L)
        xr = np.zeros((S, 3, BLOC, L + 2), f32)
        xr[:, 0, :, 1 : L + 1] = xc          # tap0: x[l-1] at col l
        xr[:, 1, :, 0:L] = xc                # tap1: x[l]
        xr[:, 2, :, 0 : L - 1] = xc[:, :, 1:L]  # tap2: x[l+1]
        maps.append({"xr": xr.astype(bf), "ones": ones, **wd})
    return maps


last_hw_ns = None
last_stage_ns = None


def _run(nc, maps, label):
    trace = bool(int(os.environ.get("BASSK_TRACE", "0")))
    res = run_bass_kernel_spmd(nc, maps, core_ids=CORE_IDS, trace=trace)
    if trace:
        global last_stage_ns
        if last_stage_ns is None:
            last_stage_ns = {}
        last_stage_ns[label] = res.exec_time_ns
    return res


def kernel(**inputs):
    global last_hw_ns, last_stage_ns
    last_stage_ns = None
    f32 = np.float32
    x = np.asarray(inputs["x"], f32)
    conv_w = np.asarray(inputs["conv_w"], f32)
    conv_b = np.asarray(inputs["conv_b"], f32)
    w1 = np.asarray(inputs["w1"], f32)
    b1 = np.asarray(inputs["b1"], f32)
    g1, be1 = np.asarray(inputs["g1"], f32), np.asarray(inputs["be1"], f32)
    m1, v1 = np.asarray(inputs["m1"], f32), np.asarray(inputs["v1"], f32)
    w2 = np.asarray(inputs["w2"], f32)
    b2 = np.asarray(inputs["b2"], f32)
    g2, be2 = np.asarray(inputs["g2"], f32), np.asarray(inputs["be2"], f32)
    m2, v2 = np.asarray(inputs["m2"], f32), np.asarray(inputs["v2"], f32)
    w3 = np.asarray(inputs["w3"], f32)
    b3 = np.asarray(inputs["b3"], f32)

    # ---- stage 1: ConvLSTM (batch-parallel)
    nc1 = _build_stage1()
    maps1 = _prep_stage1_inputs(x, conv_w, conv_b)
    res1 = _run(nc1, maps1, "stage1")
    import ml_dtypes
    bf = ml_dtypes.bfloat16
    hs_all = np.stack([res1.results[c]["hs"] for c in range(NC)])  # (8,S,H,32,L) bf16

    # ---- reshard: (8,S,H,32,L) -> flatT (S*H*L, 256), feature-major, bf16
    flatT = np.ascontiguousarray(
        hs_all.transpose(1, 2, 4, 0, 3)
    ).reshape(KTOT, B)

    # ---- stage 2: big GEMM, contract-dim sharded
    s1 = g1 / np.sqrt(v1 + EPS)
    c1 = b1 * s1 + (be1 - m1 * s1)
    w1sT = np.ascontiguousarray((w1 * s1[:, None]).T).astype(bf)    # (KTOT, 768)
    nc2 = _build_stage2()
    maps2 = []
    for c in range(NC):
        sl = slice(c * KSH, (c + 1) * KSH)
        maps2.append({
            "w1p": np.ascontiguousarray(w1sT[sl]).reshape(KCH, 128, 768),
            "ft": np.ascontiguousarray(flatT[sl]).reshape(KCH, 128, B),
        })
    res2 = _run(nc2, maps2, "stage2")
    y1 = np.sum([res2.results[c]["y1p"] for c in range(NC)], axis=0,
                dtype=np.float64).astype(f32)                       # (768, 256)

    # ---- stage 3: epilogue (batch-parallel)
    s2 = g2 / np.sqrt(v2 + EPS)
    c2 = b2 * s2 + (be2 - m2 * s2)
    cstv = np.zeros((128, 7), f32)
    cstv[:, 0:6] = c1.reshape(6, 128).T
    cstv[0:12, 6] = c2
    cstw = np.zeros((128, 82), f32)
    cstw[:, 0:72] = (w2 * s2[:, None]).T.reshape(6, 128, 12).transpose(
        1, 0, 2).reshape(128, 72)
    cstw[0:13, 72:82] = np.concatenate([w3.T, b3[None, :]], axis=0)
    nc3 = _build_stage3()
    maps3 = []
    for c in range(NC):
        ysl = np.ascontiguousarray(
            y1[:, c * BLOC : (c + 1) * BLOC].reshape(6, 128, BLOC)
            .transpose(1, 0, 2)
        )
        maps3.append({"y1s": ysl, "cstv": cstv, "cstw": cstw})
    res3 = _run(nc3, maps3, "stage3")
    y3 = np.concatenate([res3.results[c]["y3p"] for c in range(NC)], axis=0)
    if last_stage_ns and all(v is not None for v in last_stage_ns.values()):
        last_hw_ns = sum(last_stage_ns.values())
    return np.ascontiguousarray(y3, f32)



# revision 34
# speedup vs baseline: 1.0453x; 1.0453x over previous
"""Trainium2 Bass kernel for nn_CropConvLSTM.

Model: ConvLSTM (Conv1d(1+H -> 4H, k=3, pad=1), S=12 steps) over x (B=256,
S=12, L=128), then head Linear(98304->768)+BN+ReLU, Linear(768->12)+BN+ReLU,
Linear(12->10).

Distribution over 8 NeuronCores, three launches:
  Stage 1: ConvLSTM, data-parallel over batch (32 samples/core). Conv done as
    3 shifted fp32r matmuls (K=66: 64 h-channels + x-row + ones-row for the
    conv bias) accumulating in PSUM; gates on ACT/DVE/GPSIMD with all tensors
    at legal partition bases (tanh computed as 2*sigmoid(2x)-1 so a single
    per-partition-scaled sigmoid covers the [o;g] psum tile).
  Stage 2: y1 = flat @ (w1*bn1_scale).T, sharded over the 98304 contract dim
    (12288 features/core); each core emits a partial (768, 256), host reduces.
  Stage 3: bias+ReLU, Linear2+BN+ReLU, Linear3 (+b3 via ones-row trick),
    data-parallel over batch again.

BN (eval mode) is folded into the weights/biases on the host.
"""
import os
import sys

sys.path.insert(0, "/opt/trn_rl_repo")

from functools import lru_cache

import numpy as np

import concourse.bass as bass
import concourse.tile as tile
from concourse import bacc, mybir
from concourse.bass_utils import run_bass_kernel_spmd

F32 = mybir.dt.float32
F32R = mybir.dt.float32r
BF16 = mybir.dt.bfloat16
AF = mybir.ActivationFunctionType

B, S, L, H, C = 256, 12, 128, 64, 10
NC = 8
BLOC = B // NC            # 32 samples per core in stages 1/3
KTOT = S * H * L          # 98304
KSH = KTOT // NC          # 12288 contract features per core in stage 2
KCH = KSH // 128          # 96 k-chunks per core
EPS = 1e-5
CORE_IDS = list(range(NC))


# ---------------------------------------------------------------- stage 1
#
# Layout (per core, 32 samples):
#   T1 [128, 32, L+2] bf16: rows 0:64  = h stored at col j+1 (tap0 window)
#                           rows 64:128= h stored at col j   (tap1 window)
#   T2 [68, 32, L+2]  bf16: rows 0:64  = h stored at col j-1 (tap2 window)
#                           rows 64:67 = x at shifts +1/0/-1, row 67 = ones
#   Conv per step = TWO matmul passes per (quarter, gate-pair):
#     pass A: K=128, rhs=T1[:, q, 0:L]   (h taps 0+1)
#     pass B: K=68,  rhs=T2[:, q, 0:L]   (h tap2 + x*3 + bias)
#   Shifted h copies are SBUF->SBUF DMAs off the critical engines.
#   Gates: sigmoid drains on ACT ([f;i] plain, [o;g] with per-partition
#   scale [1;2]), tanh(g) unfold 2s-1 on Pool, state math on DVE in bf16
#   (2x mode), tanh(c) native on ACT over both halves packed 128-part.
QS = 8                    # samples per quarter
NQ = BLOC // QS           # 4 quarters
HB = BLOC // 2            # 16 samples per half (c packing)


@lru_cache(maxsize=1)
def _build_stage1():
    nc = bacc.Bacc("TRN2", target_bir_lowering=False, debug=False, num_devices=NC)
    xr = nc.dram_tensor("xr", [S, 3, BLOC, L + 2], BF16, kind="ExternalInput").ap()
    ones = nc.dram_tensor("ones", [1, BLOC, L + 2], BF16, kind="ExternalInput").ap()
    # gate column order per half: f/o at the half's c-partition base, i/g at
    # the other (HW requires equal base partitions for SBUF x SBUF tensor ops)
    wabs = {
        nm: nc.dram_tensor(nm, [dim, 128], BF16, kind="ExternalInput").ap()
        for hf in range(2)
        for nm, dim in ((f"wa_fi{hf}", 128), (f"wb_fi{hf}", 68),
                        (f"wa_og{hf}", 128), (f"wb_og{hf}", 68))
    }
    hs = nc.dram_tensor("hs", [S, H, BLOC, L], BF16, kind="ExternalOutput").ap()

    with tile.TileContext(nc) as tc:
        with (
            tc.tile_pool(name="persist", bufs=1) as pp,
            tc.tile_pool(name="sig", bufs=6) as sp,
            tc.tile_pool(name="tmp", bufs=6) as tp,
            tc.tile_pool(name="psfi", bufs=2, space="PSUM") as ps_fi,
            tc.tile_pool(name="psog", bufs=2, space="PSUM") as ps_og,
        ):
            t1h = pp.tile([128, BLOC, L + 2], BF16)
            # T2 is double-buffered by step parity: x rows for step s+1
            # prefetch into the other buffer, and the h tap2 copy of step s
            # goes into the buffer used by step s+1. This keeps the per-step
            # x DMA off the inter-step critical path.
            t2hs = [pp.tile([68, BLOC, L + 2], BF16, name=f"t2h{i}")
                    for i in range(2)]
            cpk = pp.tile([128, HB, L], BF16)           # c: [half0 | half1]
            wts = {}
            for hf in range(2):
                for nm, dim in ((f"wa_fi{hf}", 128), (f"wb_fi{hf}", 68),
                                (f"wa_og{hf}", 128), (f"wb_og{hf}", 68)):
                    wts[nm] = pp.tile([dim, 128], BF16, name=f"t_{nm}")
            scv0 = pp.tile([128, 1], F32)               # og act scale [1;2]
            scv1 = pp.tile([128, 1], F32)               # og act scale [2;1]

            nc.vector.memset(t1h, 0.0)
            nc.vector.memset(t2hs[0], 0.0)
            nc.vector.memset(t2hs[1], 0.0)
            nc.vector.memset(cpk, 0.0)
            nc.vector.memset(scv0[0:64], 1.0)
            nc.vector.memset(scv0[64:128], 2.0)
            nc.vector.memset(scv1[0:64], 2.0)
            nc.vector.memset(scv1[64:128], 1.0)
            nc.sync.dma_start(out=t2hs[0][67:68], in_=ones)
            nc.sync.dma_start(out=t2hs[1][67:68], in_=ones)
            for nm, tl in wts.items():
                nc.sync.dma_start(out=tl, in_=wabs[nm])
            nc.sync.dma_start(out=t2hs[0][64:67], in_=xr[0])

            # --- software pipeline helpers -------------------------------
            def front(s, q, so_tiles):
                """matmuls + gate sigmoids + c update for one quarter.

                For half hf, f and o land at partitions [64hf:64hf+64] (the
                half's c base: "pri"), i and g at the other 64 ("alt") --
                SBUF x SBUF tensor ops need equal input base partitions.
                """
                t2h = t2hs[s % 2]
                hf, qs = q // 2, (q % 2) * QS
                pri = slice(64 * hf, 64 * hf + 64)
                alt = slice(64 - 64 * hf, 128 - 64 * hf)
                smp = slice(q * QS, q * QS + QS)
                pfi = ps_fi.tile([128, QS, L], F32, name="pfi")
                pog = ps_og.tile([128, QS, L], F32, name="pog")
                # og first everywhere: it feeds the long chain
                # og -> tanh(g) -> t2 -> c -> tanh(c) -> h
                # matmul outputs must fit one PSUM bank (512 f32), so each
                # pass is issued per 4-sample half of the 8-sample tile
                for G, ps in (("og", pog), ("fi", pfi)):
                    wa, wb = wts[f"wa_{G}{hf}"], wts[f"wb_{G}{hf}"]
                    for hb in range(2):
                        sm4 = slice(q * QS + 4 * hb, q * QS + 4 * hb + 4)
                        ob = ps[:, 4 * hb : 4 * hb + 4, :]
                        nc.tensor.matmul(ob, lhsT=wa,
                                         rhs=t1h[:, sm4, 0:L],
                                         start=True, stop=False)
                        nc.tensor.matmul(ob, lhsT=wb,
                                         rhs=t2h[:, sm4, 0:L],
                                         start=False, stop=True)

                sg = sp.tile([128, QS, L], BF16, name="sg")
                # so stays f32: the 2s-1 unfold would amplify bf16 rounding
                # of sig(2g) into absolute tanh(g) error
                so = sp.tile([128, QS, L], F32, name="so")
                so_tiles[q] = so
                tg = tp.tile([128, QS, L], BF16, name="tg")
                # so = [sig(o)@pri; sig(2g)@alt], sg = [sig(f)@pri; sig(i)@alt]
                nc.scalar.activation(so, pog, AF.Sigmoid,
                                     scale=scv0 if hf == 0 else scv1)
                nc.scalar.activation(sg, pfi, AF.Sigmoid)
                # tanh(g) = 2*sig(2g) - 1  (Pool engine)
                nc.gpsimd.tensor_scalar(
                    out=tg[alt], in0=so[alt], scalar1=2.0, scalar2=-1.0,
                    op0=mybir.AluOpType.mult, op1=mybir.AluOpType.add,
                )
                # c = sig(f)*c + sig(i)*tanh(g)   (DVE, bf16 2x)
                cq = cpk[pri, qs : qs + QS, :]
                t1t = tp.tile([128, QS, L], BF16, name="t1t")
                t2t = tp.tile([128, QS, L], BF16, name="t2t")
                nc.vector.tensor_mul(t1t[pri], sg[pri], cq)
                nc.vector.tensor_mul(t2t[pri], sg[alt], tg[alt])
                nc.vector.tensor_add(cq, t1t[pri], t2t[pri])

            def tail(s, pair, so_tiles):
                """tanh(c), h writes, shifted copies and hs store."""
                t2n = t2hs[(s + 1) % 2]      # tap2 copy feeds step s+1
                qs = pair * QS
                tcs = tp.tile([128, QS, L], BF16, name="tcs")
                nc.scalar.activation(tcs, cpk[:, qs : qs + QS, :], AF.Tanh)
                for q in (pair, pair + 2):
                    hf = q // 2
                    pri = slice(64 * hf, 64 * hf + 64)
                    smp = slice(q * QS, q * QS + QS)
                    soq = so_tiles[q]
                    # h = sig(o) * tanh(c) -> T1 rows 0:64 at col shift +1
                    nc.vector.tensor_mul(
                        t1h[0:64, smp, 1 : L + 1], soq[pri], tcs[pri],
                    )
                    # tap1 copy first: it gates the next step's pass A
                    nc.sync.dma_start(
                        out=t1h[64:128, smp, 0:L],
                        in_=t1h[0:64, smp, 1 : L + 1],
                    )
                for q in (pair, pair + 2):
                    smp = slice(q * QS, q * QS + QS)
                    nc.sync.dma_start(
                        out=t2n[0:64, smp, 0:L],
                        in_=t1h[0:64, smp, 2 : L + 2],
                    )
                    nc.sync.dma_start(
                        out=hs[s, :, smp, :],
                        in_=t1h[0:64, smp, 1 : L + 1],
                    )

            # pair-slot pipeline: tail of the previous pair issues BETWEEN
            # the two quarter-fronts of the current pair, so every engine
            # stream stays dependency-ordered without head blocking
            pending = None
            for s in range(S):
                for pair in range(2):
                    so_tiles = {}
                    front(s, pair, so_tiles)
                    if pending is not None:
                        tail(*pending)
                    front(s, pair + 2, so_tiles)
                    pending = (s, pair, so_tiles)
                    if pair == 0 and s + 1 < S:
                        # prefetch next step's x rows into the other T2
                        # buffer (issued late: its WAR wait on this step's
                        # pass-B reads must not head-block the DMA queue)
                        nc.sync.dma_start(out=t2hs[(s + 1) % 2][64:67],
                                          in_=xr[s + 1])
            tail(*pending)
    nc.compile()
    return nc


# ---------------------------------------------------------------- stage 2
@lru_cache(maxsize=1)
def _build_stage2():
    nc = bacc.Bacc("TRN2", target_bir_lowering=False, debug=False, num_devices=NC)
    w1p = nc.dram_tensor("w1p", [KCH, 128, 768], BF16, kind="ExternalInput").ap()
    ft = nc.dram_tensor("ft", [KCH, 128, B], BF16, kind="ExternalInput").ap()
    y1p = nc.dram_tensor("y1p", [768, B], F32, kind="ExternalOutput").ap()

    KB = 3                       # k-chunks per DMA batch
    NB = KCH // KB               # 48 batches

    with tile.TileContext(nc) as tc:
        with (
            tc.tile_pool(name="wp", bufs=4) as wp,
            tc.tile_pool(name="rp", bufs=4) as rp,
            tc.tile_pool(name="op", bufs=2) as op,
            tc.tile_pool(name="ps", bufs=1, space="PSUM") as ps,
        ):
            acc = [ps.tile([128, B], F32, name=f"acc{m}") for m in range(6)]
            for kb in range(NB):
                wt = wp.tile([128, KB, 768], BF16, name="wt")
                rt = rp.tile([128, KB, B], BF16, name="rt")
                nc.sync.dma_start(
                    out=wt,
                    in_=w1p[kb * KB : (kb + 1) * KB].rearrange("k p m -> p k m"),
                )
                nc.gpsimd.dma_start(
                    out=rt,
                    in_=ft[kb * KB : (kb + 1) * KB].rearrange("k p b -> p k b"),
                )
                last_kb = kb == NB - 1
                for kc in range(KB):
                    last = last_kb and kc == KB - 1
                    for m in range(6):
                        nc.tensor.matmul(
                            acc[m], lhsT=wt[:, kc, m * 128 : (m + 1) * 128],
                            rhs=rt[:, kc, :],
                            start=(kb == 0 and kc == 0),
                            stop=last,
                        )
                        if last:
                            # drain this M-tile immediately after its final
                            # accumulation to overlap the output tail
                            ot = op.tile([128, B], F32, name="ot")
                            nc.vector.tensor_copy(ot, acc[m])
                            nc.sync.dma_start(
                                out=y1p[m * 128 : (m + 1) * 128], in_=ot,
                            )
    nc.compile()
    return nc


# ---------------------------------------------------------------- stage 3
# One packed const tensor, ReLUs on DVE (no ACT table load), 2 input DMAs.
# cst cols: 0:6 = c1 (bias1, transposed 6x128), 6:78 = w2 (6 k-chunks of
# (128,12)), 78:79 = c2 (partitions 0:12), 79:89 = [w3.T; b3] (parts 0:13)
@lru_cache(maxsize=1)
def _build_stage3():
    nc = bacc.Bacc("TRN2", target_bir_lowering=False, debug=False, num_devices=NC)
    y1s = nc.dram_tensor("y1s", [128, 6, BLOC], F32R, kind="ExternalInput").ap()
    cstv = nc.dram_tensor("cstv", [128, 7], F32, kind="ExternalInput").ap()
    cstw = nc.dram_tensor("cstw", [128, 82], F32R, kind="ExternalInput").ap()
    y3p = nc.dram_tensor("y3p", [BLOC, C], F32, kind="ExternalOutput").ap()
    MX = mybir.AluOpType.max
    AD = mybir.AluOpType.add

    with tile.TileContext(nc) as tc:
        with (
            tc.tile_pool(name="sb", bufs=1) as sb,
            tc.tile_pool(name="ps", bufs=1, space="PSUM") as ps,
        ):
            yt = sb.tile([128, 6, BLOC], F32R)
            cv = sb.tile([128, 7], F32)
            cw = sb.tile([128, 82], F32R)
            nc.sync.dma_start(out=cv, in_=cstv)
            nc.sync.dma_start(out=cw, in_=cstw)
            nc.sync.dma_start(out=yt, in_=y1s)

            r1 = sb.tile([128, 6, BLOC], F32R)
            p2 = ps.tile([12, BLOC], F32)
            for kc in range(6):
                # r1 = relu(y1 + c1)  (DVE, per-partition bias)
                nc.vector.tensor_scalar(
                    out=r1[:, kc, :], in0=yt[:, kc, :],
                    scalar1=cv[:, kc : kc + 1], scalar2=0.0, op0=AD, op1=MX,
                )
                nc.tensor.matmul(
                    p2, lhsT=cw[:, 12 * kc : 12 * kc + 12],
                    rhs=r1[:, kc, :], start=(kc == 0), stop=(kc == 5),
                )
            r2 = sb.tile([13, BLOC], F32R)
            # ones row lives at partition 12 (not 32-aligned): fill the whole
            # tile with 1.0 first, then overwrite rows 0..11
            nc.vector.memset(r2.bitcast(F32), 1.0)
            nc.vector.tensor_scalar(
                out=r2[0:12], in0=p2, scalar1=cv[0:12, 6:7], scalar2=0.0,
                op0=AD, op1=MX,
            )
            p3 = ps.tile([BLOC, C], F32)
            nc.tensor.matmul(p3, lhsT=r2, rhs=cw[0:13, 72:82],
                             start=True, stop=True)
            ot = sb.tile([BLOC, C], F32)
            nc.vector.tensor_copy(ot, p3)
            nc.sync.dma_start(out=y3p, in_=ot)
    nc.compile()
    return nc


# ---------------------------------------------------------------- host glue
def _prep_stage1_inputs(x, conv_w, conv_b):
    """Per-core stage-1 in_maps. conv_w: (4H, 1+H, 3), in-ch 0 = x, 1.. = h.
    Gate order in conv_w rows: i, f, o, g (64 each). Our gate tiles are
    [f; i] and [o; g]. lhsT layouts:
      wa_G [128, 128]: K rows 0:64 = h tap0, 64:128 = h tap1
      wb_G [68, 128]:  K rows 0:64 = h tap2, 64:67 = x taps 0/1/2, 67 = bias
    """
    import ml_dtypes
    bf = ml_dtypes.bfloat16
    f32 = np.float32
    w = np.asarray(conv_w, f32)
    b = np.asarray(conv_b, f32)
    gi, gf, go, gg = w[0:64], w[64:128], w[128:192], w[192:256]
    bi, bf_, bo, bg = b[0:64], b[64:128], b[128:192], b[192:256]

    def lhst(wA, wB, bA, bB):
        wa = np.zeros((128, 128), f32)
        wb = np.zeros((68, 128), f32)
        for col0, wg_, bg_ in ((0, wA, bA), (64, wB, bB)):
            # wg_: (64 out-ch, 65 in-ch, 3 taps); h = in-ch 1.., x = in-ch 0
            wa[0:64, col0 : col0 + 64] = wg_[:, 1:65, 0].T
            wa[64:128, col0 : col0 + 64] = wg_[:, 1:65, 1].T
            wb[0:64, col0 : col0 + 64] = wg_[:, 1:65, 2].T
            wb[64:67, col0 : col0 + 64] = wg_[:, 0, :].T
            wb[67, col0 : col0 + 64] = bg_
        return wa.astype(bf), wb.astype(bf)

    wd = {}
    # half 0: [f|i], [o|g]; half 1: [i|f], [g|o] (f/o at the half's c base)
    wd["wa_fi0"], wd["wb_fi0"] = lhst(gf, gi, bf_, bi)
    wd["wa_og0"], wd["wb_og0"] = lhst(go, gg, bo, bg)
    wd["wa_fi1"], wd["wb_fi1"] = lhst(gi, gf, bi, bf_)
    wd["wa_og1"], wd["wb_og1"] = lhst(gg, go, bg, bo)

    ones = np.ones((1, BLOC, L + 2), f32).astype(bf)
    maps = []
    for c in range(NC):
        xc = np.asarray(x[c * BLOC : (c + 1) * BLOC].transpose(1, 0, 2), f32)
        # xr[s]: rows at shifts +1 / 0 / -1 over L+2 cols (window = cols 0:L)
        xr = np.zeros((S, 3, BLOC, L + 2), f32)
        xr[:, 0, :, 1 : L + 1] = xc          # tap0: x[l-1] at col l
        xr[:, 1, :, 0:L] = xc                # tap1: x[l]
        xr[:, 2, :, 0 : L - 1] = xc[:, :, 1:L]  # tap2: x[l+1]
        maps.append({"xr": xr.astype(bf), "ones": ones, **wd})
    return maps


last_hw_ns = None
last_stage_ns = None


def _run(nc, maps, label):
    trace = bool(int(os.environ.get("BASSK_TRACE", "0")))
    res = run_bass_kernel_spmd(nc, maps, core_ids=CORE_IDS, trace=trace)
    if trace:
        global last_stage_ns
        if last_stage_ns is None:
            last_stage_ns = {}
        last_stage_ns[label] = res.exec_time_ns
    return res


def kernel(**inputs):
    global last_hw_ns, last_stage_ns
    last_stage_ns = None
    f32 = np.float32
    x = np.asarray(inputs["x"], f32)
    conv_w = np.asarray(inputs["conv_w"], f32)
    conv_b = np.asarray(inputs["conv_b"], f32)
    w1 = np.asarray(inputs["w1"], f32)
    b1 = np.asarray(inputs["b1"], f32)
    g1, be1 = np.asarray(inputs["g1"], f32), np.asarray(inputs["be1"], f32)
    m1, v1 = np.asarray(inputs["m1"], f32), np.asarray(inputs["v1"], f32)
    w2 = np.asarray(inputs["w2"], f32)
    b2 = np.asarray(inputs["b2"], f32)
    g2, be2 = np.asarray(inputs["g2"], f32), np.asarray(inputs["be2"], f32)
    m2, v2 = np.asarray(inputs["m2"], f32), np.asarray(inputs["v2"], f32)
    w3 = np.asarray(inputs["w3"], f32)
    b3 = np.asarray(inputs["b3"], f32)

    # ---- stage 1: ConvLSTM (batch-parallel)
    nc1 = _build_stage1()
    maps1 = _prep_stage1_inputs(x, conv_w, conv_b)
    res1 = _run(nc1, maps1, "stage1")
    import ml_dtypes
    bf = ml_dtypes.bfloat16
    hs_all = np.stack([res1.results[c]["hs"] for c in range(NC)])  # (8,S,H,32,L) bf16

    # ---- reshard: (8,S,H,32,L) -> flatT (S*H*L, 256), feature-major, bf16
    flatT = np.ascontiguousarray(
        hs_all.transpose(1, 2, 4, 0, 3)
    ).reshape(KTOT, B)

    # ---- stage 2: big GEMM, contract-dim sharded
    s1 = g1 / np.sqrt(v1 + EPS)
    c1 = b1 * s1 + (be1 - m1 * s1)
    w1sT = np.ascontiguousarray((w1 * s1[:, None]).T).astype(bf)    # (KTOT, 768)
    nc2 = _build_stage2()
    maps2 = []
    for c in range(NC):
        sl = slice(c * KSH, (c + 1) * KSH)
        maps2.append({
            "w1p": np.ascontiguousarray(w1sT[sl]).reshape(KCH, 128, 768),
            "ft": np.ascontiguousarray(flatT[sl]).reshape(KCH, 128, B),
        })
    res2 = _run(nc2, maps2, "stage2")
    y1 = np.sum([res2.results[c]["y1p"] for c in range(NC)], axis=0,
                dtype=np.float64).astype(f32)                       # (768, 256)

    # ---- stage 3: epilogue (batch-parallel)
    s2 = g2 / np.sqrt(v2 + EPS)
    c2 = b2 * s2 + (be2 - m2 * s2)
    cstv = np.zeros((128, 7), f32)
    cstv[:, 0:6] = c1.reshape(6, 128).T
    cstv[0:12, 6] = c2
    cstw = np.zeros((128, 82), f32)
    cstw[:, 0:72] = (w2 * s2[:, None]).T.reshape(6, 128, 12).transpose(
        1, 0, 2).reshape(128, 72)
    cstw[0:13, 72:82] = np.concatenate([w3.T, b3[None, :]], axis=0)
    nc3 = _build_stage3()
    maps3 = []
    for c in range(NC):
        ysl = np.ascontiguousarray(
            y1[:, c * BLOC : (c + 1) * BLOC].reshape(6, 128, BLOC)
            .transpose(1, 0, 2)
        )
        maps3.append({"y1s": ysl, "cstv": cstv, "cstw": cstw})
    res3 = _run(nc3, maps3, "stage3")
    y3 = np.concatenate([res3.results[c]["y3p"] for c in range(NC)], axis=0)
    if last_stage_ns and all(v is not None for v in last_stage_ns.values()):
        last_hw_ns = sum(last_stage_ns.values())
    return np.ascontiguousarray(y3, f32)

